# revision 18
# baseline (speedup 1.0000x reference)
"""Multi-head causal attention (scores = K @ Q^T variant) on 8 TRN2 NeuronCores.

Head-parallel sharding: core c computes heads (2c, 2c+1) end-to-end and the
host concatenates the per-core [T, 128] outputs along the feature axis.

Per-core kernel layout notes:
  - Host passes x transposed ([D, T]) and pre-cast to bf16 so every
    projection matmul has the contraction dim (d) on SBUF partitions with
    zero on-device transposes.
  - Q^T/K^T are stored [128, T] with head0 on partitions 0-63 and head1 on
    64-127, letting the S^T matmuls for both heads run concurrently on
    disjoint PE row-groups (tile_position).
  - Scores are computed transposed (S^T[j, i] = Q_j . K_i) so that the AV
    contraction (over j) lands on the partition axis with no transposes.
  - V is stored in natural layout with a fused ones-column ([V | 1]) so a
    single AV matmul produces both the weighted sum and the softmax
    denominator (PSUM row 64).
  - Softmax skips the max-subtraction (scores are ~N(0,1); exp is safe in
    fp32) which matches jax.nn.softmax up to rounding.
  - Matmul operands are bf16 (1 cycle/row on the PE; fp32 streams at ~2);
    PSUM accumulation and the final normalization stay fp32.
  - Projection chunk k and attention i-block k are emitted interleaved
    (i-block k only needs x columns < 512*(k+1)) from one pool set sized to
    exactly 8 PSUM banks, so projections and attention overlap instead of
    serializing on a pool boundary.
"""

import numpy as np

T, D, H, HS = 4096, 1024, 16, 64
NCORES = 8
HPC = H // NCORES  # heads per core = 2
DC = D // 128      # 8 contraction chunks
TC = T // 512      # 8 t-chunks for projections
IB = T // 512      # 8 i-blocks (512 output rows each)
JBN = T // 128     # 32 j-blocks (128 keys each)

_cached_nc = None


def _emit(tc, nc, xT, w6, out):
    import concourse.bass as bass  # noqa: F401
    import concourse.mybir as mybir

    f32 = mybir.dt.float32
    bf16 = mybir.dt.bfloat16
    Exp = mybir.ActivationFunctionType.Exp
    ne = mybir.AluOpType.not_equal
    ge = mybir.AluOpType.is_ge

    with (
        tc.tile_pool(name="const", bufs=1) as constp,
        tc.tile_pool(name="wpool", bufs=1) as wpool,
        tc.tile_pool(name="bigp", bufs=1) as bigp,
        tc.tile_pool(name="xpool", bufs=3) as xpool,
        tc.tile_pool(name="vtp", bufs=2) as vtp,
        tc.tile_pool(name="esp", bufs=6) as esp,
        tc.tile_pool(name="finp", bufs=4) as finp,
        # PSUM budget (8 banks total): s 2x2 + o 2x1 + p 1 + t 1.
        tc.tile_pool(name="sp", bufs=2, space="PSUM") as sp,
        tc.tile_pool(name="op", bufs=2, space="PSUM") as op,
        tc.tile_pool(name="pp", bufs=1, space="PSUM") as pp,
        tc.tile_pool(name="tp", bufs=1, space="PSUM") as tp,
    ):
        # ---- input DMAs for weights + first x chunk go first ------------
        w6sb = wpool.tile([128, DC, 6 * HS], bf16)
        xts = []
        xt0 = xpool.tile([128, DC, 512], bf16, tag="xt", name="xt0")
        for dc in range(DC):
            nc.sync.dma_start(out=w6sb[:, dc, :], in_=w6[dc * 128:(dc + 1) * 128, :])
            nc.sync.dma_start(out=xt0[:, dc, :], in_=xT[dc * 128:(dc + 1) * 128, 0:512])
        xts.append(xt0)

        # ---- constants (gpsimd; overlaps the DMAs) ----------------------
        # id64: two stacked 64x64 identities so both head slices (partition
        # offset 0 and 64) see an identity for the V transposes.
        id64 = constp.tile([128, 64], bf16)
        nc.gpsimd.memset(id64, 0.0)
        nc.gpsimd.affine_select(
            out=id64, in_=id64, compare_op=ne, fill=1.0,
            base=0, channel_multiplier=1, pattern=[[-1, 64]],
        )
        nc.gpsimd.affine_select(
            out=id64, in_=id64, compare_op=ne, fill=1.0,
            base=-64, channel_multiplier=1, pattern=[[-1, 64]],
        )
        # Causal masks for the 4 diagonal block offsets: keep iff il >= jl + 128*q.
        mask4 = constp.tile([128, 4, 512], bf16)
        for q in range(4):
            nc.gpsimd.memset(mask4[:, q, :], 1.0)
            nc.gpsimd.affine_select(
                out=mask4[:, q, :], in_=mask4[:, q, :], compare_op=ge, fill=0.0,
                base=-128 * q, channel_multiplier=-1, pattern=[[1, 512]],
            )

        # ---- persistent activations ------------------------------------
        QT = bigp.tile([128, T], bf16)   # head0 rows 0-63, head1 rows 64-127
        KT = bigp.tile([128, T], bf16)
        Vext0 = bigp.tile([128, JBN, HS + 1], bf16)
        Vext1 = bigp.tile([128, JBN, HS + 1], bf16)
        onesb = constp.tile([128, JBN], bf16)
        nc.gpsimd.memset(onesb, 1.0)
        nc.vector.tensor_copy(Vext0[:, :, HS], onesb)
        nc.vector.tensor_copy(Vext1[:, :, HS], onesb)

        def emit_proj_chunk(tcj):
            ts = slice(tcj * 512, (tcj + 1) * 512)
            if tcj + 1 < TC:  # prefetch next x chunk
                nxt = slice((tcj + 1) * 512, (tcj + 2) * 512)
                xtn = xpool.tile([128, DC, 512], bf16, tag="xt", name=f"xt{tcj + 1}")
                for dc in range(DC):
                    nc.sync.dma_start(
                        out=xtn[:, dc, :], in_=xT[dc * 128:(dc + 1) * 128, nxt]
                    )
                xts.append(xtn)
            xt = xts[tcj]
            for fc, dest in ((0, QT), (1, KT)):
                ps = pp.tile([128, 512], f32, tag="p", name=f"ps{fc}_{tcj}")
                for dc in range(DC):
                    nc.tensor.matmul(
                        ps,
                        lhsT=w6sb[:, dc, fc * 128:(fc + 1) * 128],
                        rhs=xt[:, dc, :],
                        start=(dc == 0), stop=(dc == DC - 1),
                    )
                nc.vector.tensor_copy(dest[:, ts], ps)
            psv = pp.tile([128, 512], f32, tag="p", name=f"psv_{tcj}")
            for dc in range(DC):
                nc.tensor.matmul(
                    psv,
                    lhsT=w6sb[:, dc, 256:384],
                    rhs=xt[:, dc, :],
                    start=(dc == 0), stop=(dc == DC - 1),
                )
            vts = vtp.tile([128, 512], bf16, tag="vts", name=f"vts_{tcj}")
            nc.vector.tensor_copy(vts, psv)
            for q in range(4):
                for h in range(HPC):
                    vdst = Vext0 if h == 0 else Vext1
                    ptv = tp.tile([128, 64], bf16, tag="t", name=f"ptv{h}_{tcj}_{q}")
                    nc.tensor.transpose(
                        ptv,
                        in_=vts[h * 64:(h + 1) * 64, q * 128:(q + 1) * 128],
                        identity=id64[h * 64:(h + 1) * 64, :],
                        tile_position=(h * 64, 0),
                    )
                    nc.vector.tensor_copy(vdst[:, tcj * 4 + q, 0:HS], ptv)

        def emit_attn_block(ib):
            isl = slice(ib * 512, (ib + 1) * 512)
            njb = 4 * (ib + 1)
            po = [
                op.tile([65, 512], f32, tag="o", name=f"po{h}_{ib}")
                for h in range(HPC)
            ]
            pending = None
            for jb in range(njb):
                ps = sp.tile([128, 2, 512], f32, tag="s", name=f"s_{ib}_{jb}")
                for h in range(HPC):
                    nc.tensor.matmul(
                        ps[:, h, :],
                        lhsT=QT[h * 64:(h + 1) * 64, jb * 128:(jb + 1) * 128],
                        rhs=KT[h * 64:(h + 1) * 64, isl],
                        start=True, stop=True,
                        tile_position=(h * 64, 0),
                    )
                es = esp.tile([128, 2, 512], bf16, tag="es", name=f"es_{ib}_{jb}")
                nc.scalar.activation(es, ps, Exp, scale=float(1.0 / np.sqrt(HS)))
                q = jb - 4 * ib
                if q >= 0:  # diagonal block: zero out j > i entries
                    for h in range(HPC):
                        nc.vector.tensor_mul(es[:, h, :], es[:, h, :], mask4[:, q, :])
                if pending is not None:
                    pjb, pes = pending
                    for h in range(HPC):
                        nc.tensor.matmul(
                            po[h],
                            lhsT=Vext0[:, pjb, :] if h == 0 else Vext1[:, pjb, :],
                            rhs=pes[:, h, :],
                            start=(pjb == 0), stop=False,
                        )
                pending = (jb, es)
            pjb, pes = pending
            for h in range(HPC):
                nc.tensor.matmul(
                    po[h],
                    lhsT=Vext0[:, pjb, :] if h == 0 else Vext1[:, pjb, :],
                    rhs=pes[:, h, :],
                    start=(pjb == 0), stop=True,
                )
            # evacuate the unnormalized O^T + denominator row; the host does
            # the (tiny) divide and the un-transpose during the gather.
            for h in range(HPC):
                ot = finp.tile([65, 512], f32, tag="ot", name=f"ot{h}_{ib}")
                nc.vector.tensor_copy(ot, po[h])
                nc.sync.dma_start(out=out[h * 65:(h + 1) * 65, isl], in_=ot)

        # Staircase: attention block k only depends on projection chunks <= k.
        for k in range(TC):
            emit_proj_chunk(k)
            emit_attn_block(k)


# walrus engine-instruction encodings have a single sync-wait slot; hoist
# extra waits onto per-wait NoOps for everything except generated NoOps.
_NO_HOIST_TYPES = frozenset({"InstNoOp"})


def _pair_ldweights(nc):
    """Reorder LDW0,MM0,LDW1,MM1 -> LDW0,LDW1,MM0,MM1 for row-group pairs.

    When the second weight load targets PE rows 64-127 while the first
    matmul only occupies rows 0-63, the loads run concurrently on disjoint
    sub-arrays and both matmul streams overlap, instead of serializing the
    second load behind the first stream.
    """
    for f in nc.m.functions:
        for blk in f.blocks:
            insts = blk.instructions
            changed = False
            i = 0
            while i + 3 < len(insts):
                a, b, c, d = insts[i:i + 4]
                if (
                    type(a).__name__ == "InstLdweights"
                    and type(b).__name__ == "InstMatmult"
                    and type(c).__name__ == "InstLdweights"
                    and type(d).__name__ == "InstMatmult"
                    and b.tile_position is not None
                    and c.tile_position is not None
                    and b.tile_position[0] == 0
                    and c.tile_position[0] == 64
                    and b.tile_size is not None
                    and b.tile_size[0] <= 64
                ):
                    insts[i + 1], insts[i + 2] = c, b
                    changed = True
                    i += 4
                else:
                    i += 1
            if changed:
                blk.instructions = insts


def _legalize_waits(nc):
    """Hoist multi-waits off engine instructions onto preceding NoOps.

    Most walrus instruction encodings (S3_LW matmul, DMA, ACT, DVE, drain)
    only have room for a single sync-wait command; Tile freely attaches
    several. Waits execute on the engine's sequencer in program order, so
    moving them to immediately-preceding NoOps is semantics-preserving.
    """
    import bass_rust

    for f in nc.m.functions:
        for blk in f.blocks:
            out = []
            changed = False
            for inst in blk.instructions:
                si = getattr(inst, "sync_info", None)
                if (
                    type(inst).__name__ not in _NO_HOIST_TYPES
                    and si is not None
                    and len(si.on_wait) >= 2
                ):
                    waits = list(si.on_wait)
                    for k, w in enumerate(waits[:-1]):
                        nop = bass_rust.InstNoOp(name=f"{inst.name}_hoistw{k}")
                        nop.engine = inst.engine
                        nop.sync_info = bass_rust.SyncInfo(
                            on_wait=[w], on_update=[]
                        )
                        out.append(nop)
                    si.on_wait = [waits[-1]]
                    changed = True
                out.append(inst)
            if changed:
                blk.instructions = out


def _build_program():
    import concourse.bass as bass
    import concourse.mybir as mybir
    import concourse.tile as tile

    nc = bass.Bass("TRN2", target_bir_lowering=False, debug=False, num_devices=NCORES)
    xT = nc.dram_tensor("xT", [D, T], mybir.dt.bfloat16, kind="ExternalInput").ap()
    w6 = nc.dram_tensor("w6", [D, 6 * HS], mybir.dt.bfloat16, kind="ExternalInput").ap()
    out = nc.dram_tensor("outR", [HPC * (HS + 1), T], mybir.dt.float32, kind="ExternalOutput").ap()

    with tile.TileContext(nc) as tc:
        _emit(tc, nc, xT, w6, out)
    _pair_ldweights(nc)
    _legalize_waits(nc)
    return nc


def _in_maps(x, Wk, Wq, Wv):
    import ml_dtypes

    bf = ml_dtypes.bfloat16
    xTh = np.ascontiguousarray(np.asarray(x, dtype=np.float32).T.astype(bf))
    maps = []
    for c in range(NCORES):
        h0, h1 = HPC * c, HPC * c + 1
        W6 = np.concatenate(
            [Wq[h0], Wq[h1], Wk[h0], Wk[h1], Wv[h0], Wv[h1]], axis=1
        ).astype(bf)
        maps.append({"xT": xTh, "w6": np.ascontiguousarray(W6)})
    return maps


def get_program():
    global _cached_nc
    if _cached_nc is None:
        _cached_nc = _build_program()
    return _cached_nc


def kernel(x, Wk, Wq, Wv):
    from concourse.bass_utils import run_bass_kernel_spmd

    nc = get_program()
    res = run_bass_kernel_spmd(nc, _in_maps(x, Wk, Wq, Wv), core_ids=list(range(NCORES)))
    cols = []
    for c in range(NCORES):
        raw = res.results[c]["outR"]  # [2*65, T]: per head 64 rows O^T + denom
        for h in range(HPC):
            o = raw[h * 65:h * 65 + HS]
            den = raw[h * 65 + HS:h * 65 + HS + 1]
            cols.append((o / den).T)
    return np.ascontiguousarray(np.concatenate(cols, axis=1), dtype=np.float32)


# revision 19
# speedup vs baseline: 1.0053x; 1.0053x over previous
"""Multi-head causal attention (scores = K @ Q^T variant) on 8 TRN2 NeuronCores.

Head-parallel sharding: core c computes heads (2c, 2c+1) end-to-end and the
host concatenates the per-core [T, 128] outputs along the feature axis.

Per-core kernel layout notes:
  - Host passes x transposed ([D, T]) and pre-cast to bf16 so every
    projection matmul has the contraction dim (d) on SBUF partitions with
    zero on-device transposes.
  - Q^T/K^T are stored [128, T] with head0 on partitions 0-63 and head1 on
    64-127, letting the S^T matmuls for both heads run concurrently on
    disjoint PE row-groups (tile_position).
  - Scores are computed transposed (S^T[j, i] = Q_j . K_i) so that the AV
    contraction (over j) lands on the partition axis with no transposes.
  - V is stored in natural layout with a fused ones-column ([V | 1]) so a
    single AV matmul produces both the weighted sum and the softmax
    denominator (PSUM row 64).
  - Softmax skips the max-subtraction (scores are ~N(0,1); exp is safe in
    fp32) which matches jax.nn.softmax up to rounding.
  - Matmul operands are bf16 (1 cycle/row on the PE; fp32 streams at ~2);
    PSUM accumulation and the final normalization stay fp32.
  - Projection chunk k and attention i-block k are emitted interleaved
    (i-block k only needs x columns < 512*(k+1)) from one pool set sized to
    exactly 8 PSUM banks, so projections and attention overlap instead of
    serializing on a pool boundary.
"""

import numpy as np

T, D, H, HS = 4096, 1024, 16, 64
NCORES = 8
HPC = H // NCORES  # heads per core = 2
DC = D // 128      # 8 contraction chunks
TC = T // 512      # 8 t-chunks for projections
IB = T // 512      # 8 i-blocks (512 output rows each)
JBN = T // 128     # 32 j-blocks (128 keys each)

_cached_nc = None


def _emit(tc, nc, xT, w6, out):
    import concourse.bass as bass  # noqa: F401
    import concourse.mybir as mybir

    f32 = mybir.dt.float32
    bf16 = mybir.dt.bfloat16
    Exp = mybir.ActivationFunctionType.Exp
    ne = mybir.AluOpType.not_equal
    ge = mybir.AluOpType.is_ge

    with (
        tc.tile_pool(name="const", bufs=1) as constp,
        tc.tile_pool(name="wpool", bufs=1) as wpool,
        tc.tile_pool(name="bigp", bufs=1) as bigp,
        tc.tile_pool(name="xpool", bufs=3) as xpool,
        tc.tile_pool(name="vtp", bufs=2) as vtp,
        tc.tile_pool(name="esp", bufs=6) as esp,
        tc.tile_pool(name="finp", bufs=4) as finp,
        # PSUM budget (8 banks total): s 2x2 + o 2x1 + p 1 + t 1.
        tc.tile_pool(name="sp", bufs=2, space="PSUM") as sp,
        tc.tile_pool(name="op", bufs=2, space="PSUM") as op,
        tc.tile_pool(name="pp", bufs=1, space="PSUM") as pp,
        tc.tile_pool(name="tp", bufs=1, space="PSUM") as tp,
    ):
        # ---- input DMAs for weights + first x chunk go first ------------
        w6r = w6.rearrange("(dc p) f -> p dc f", p=128)
        xTr = xT.rearrange("(dc p) t -> p dc t", p=128)
        w6sb = wpool.tile([128, DC, 6 * HS], bf16)
        xts = []
        xt0 = xpool.tile([128, DC, 512], bf16, tag="xt", name="xt0")
        # interleave so the first projection's (w6 dc0, xt0 dc0) land first;
        # the first x chunk stays per-dc so matmuls start on partial data.
        nc.sync.dma_start(out=w6sb[:, 0, :], in_=w6r[:, 0, :])
        nc.sync.dma_start(out=xt0[:, 0, :], in_=xTr[:, 0, 0:512])
        nc.sync.dma_start(out=w6sb[:, 1:DC, :], in_=w6r[:, 1:DC, :])
        for dc in range(1, DC):
            nc.sync.dma_start(out=xt0[:, dc, :], in_=xTr[:, dc, 0:512])
        xts.append(xt0)

        # ---- constants (gpsimd; overlaps the DMAs) ----------------------
        # id64: two stacked 64x64 identities so both head slices (partition
        # offset 0 and 64) see an identity for the V transposes.
        id64 = constp.tile([128, 64], bf16)
        nc.gpsimd.memset(id64, 0.0)
        nc.gpsimd.affine_select(
            out=id64, in_=id64, compare_op=ne, fill=1.0,
            base=0, channel_multiplier=1, pattern=[[-1, 64]],
        )
        nc.gpsimd.affine_select(
            out=id64, in_=id64, compare_op=ne, fill=1.0,
            base=-64, channel_multiplier=1, pattern=[[-1, 64]],
        )
        # Causal masks for the 4 diagonal block offsets: keep iff il >= jl + 128*q.
        mask4 = constp.tile([128, 4, 512], bf16)
        for q in range(4):
            nc.gpsimd.memset(mask4[:, q, :], 1.0)
            nc.gpsimd.affine_select(
                out=mask4[:, q, :], in_=mask4[:, q, :], compare_op=ge, fill=0.0,
                base=-128 * q, channel_multiplier=-1, pattern=[[1, 512]],
            )

        # ---- persistent activations ------------------------------------
        QT = bigp.tile([128, T], bf16)   # head0 rows 0-63, head1 rows 64-127
        KT = bigp.tile([128, T], bf16)
        Vext0 = bigp.tile([128, JBN, HS + 1], bf16)
        Vext1 = bigp.tile([128, JBN, HS + 1], bf16)
        onesb = constp.tile([128, JBN], bf16)
        nc.gpsimd.memset(onesb, 1.0)
        nc.vector.tensor_copy(Vext0[:, :, HS], onesb)
        nc.vector.tensor_copy(Vext1[:, :, HS], onesb)

        def emit_proj_chunk(tcj):
            ts = slice(tcj * 512, (tcj + 1) * 512)
            if tcj + 1 < TC:  # prefetch next x chunk (batched 3D DMAs)
                nxt = slice((tcj + 1) * 512, (tcj + 2) * 512)
                xtn = xpool.tile([128, DC, 512], bf16, tag="xt", name=f"xt{tcj + 1}")
                nc.sync.dma_start(out=xtn[:, 0:4, :], in_=xTr[:, 0:4, nxt])
                nc.sync.dma_start(out=xtn[:, 4:8, :], in_=xTr[:, 4:8, nxt])
                xts.append(xtn)
            xt = xts[tcj]
            for fc, dest in ((0, QT), (1, KT)):
                ps = pp.tile([128, 512], f32, tag="p", name=f"ps{fc}_{tcj}")
                for dc in range(DC):
                    nc.tensor.matmul(
                        ps,
                        lhsT=w6sb[:, dc, fc * 128:(fc + 1) * 128],
                        rhs=xt[:, dc, :],
                        start=(dc == 0), stop=(dc == DC - 1),
                    )
                nc.vector.tensor_copy(dest[:, ts], ps)
            psv = pp.tile([128, 512], f32, tag="p", name=f"psv_{tcj}")
            for dc in range(DC):
                nc.tensor.matmul(
                    psv,
                    lhsT=w6sb[:, dc, 256:384],
                    rhs=xt[:, dc, :],
                    start=(dc == 0), stop=(dc == DC - 1),
                )
            vts = vtp.tile([128, 512], bf16, tag="vts", name=f"vts_{tcj}")
            nc.vector.tensor_copy(vts, psv)
            for q in range(4):
                for h in range(HPC):
                    vdst = Vext0 if h == 0 else Vext1
                    ptv = tp.tile([128, 64], bf16, tag="t", name=f"ptv{h}_{tcj}_{q}")
                    nc.tensor.transpose(
                        ptv,
                        in_=vts[h * 64:(h + 1) * 64, q * 128:(q + 1) * 128],
                        identity=id64[h * 64:(h + 1) * 64, :],
                        tile_position=(h * 64, 0),
                    )
                    nc.vector.tensor_copy(vdst[:, tcj * 4 + q, 0:HS], ptv)

        def emit_attn_block(ib):
            isl = slice(ib * 512, (ib + 1) * 512)
            njb = 4 * (ib + 1)
            po = [
                op.tile([65, 512], f32, tag="o", name=f"po{h}_{ib}")
                for h in range(HPC)
            ]
            pending = None
            for jb in range(njb):
                ps = sp.tile([128, 2, 512], f32, tag="s", name=f"s_{ib}_{jb}")
                for h in range(HPC):
                    nc.tensor.matmul(
                        ps[:, h, :],
                        lhsT=QT[h * 64:(h + 1) * 64, jb * 128:(jb + 1) * 128],
                        rhs=KT[h * 64:(h + 1) * 64, isl],
                        start=True, stop=True,
                        tile_position=(h * 64, 0),
                    )
                es = esp.tile([128, 2, 512], bf16, tag="es", name=f"es_{ib}_{jb}")
                nc.scalar.activation(es, ps, Exp, scale=float(1.0 / np.sqrt(HS)))
                q = jb - 4 * ib
                if q >= 0:  # diagonal block: zero out j > i entries
                    for h in range(HPC):
                        nc.vector.tensor_mul(es[:, h, :], es[:, h, :], mask4[:, q, :])
                if pending is not None:
                    pjb, pes = pending
                    for h in range(HPC):
                        nc.tensor.matmul(
                            po[h],
                            lhsT=Vext0[:, pjb, :] if h == 0 else Vext1[:, pjb, :],
                            rhs=pes[:, h, :],
                            start=(pjb == 0), stop=False,
                        )
                pending = (jb, es)
            pjb, pes = pending
            for h in range(HPC):
                nc.tensor.matmul(
                    po[h],
                    lhsT=Vext0[:, pjb, :] if h == 0 else Vext1[:, pjb, :],
                    rhs=pes[:, h, :],
                    start=(pjb == 0), stop=True,
                )
            # evacuate the unnormalized O^T + denominator row; the host does
            # the (tiny) divide and the un-transpose during the gather.
            for h in range(HPC):
                ot = finp.tile([65, 512], f32, tag="ot", name=f"ot{h}_{ib}")
                nc.vector.tensor_copy(ot, po[h])
                nc.sync.dma_start(out=out[h * 65:(h + 1) * 65, isl], in_=ot)

        # Staircase: attention block k only depends on projection chunks <= k.
        for k in range(TC):
            emit_proj_chunk(k)
            emit_attn_block(k)


# walrus engine-instruction encodings have a single sync-wait slot; hoist
# extra waits onto per-wait NoOps for everything except generated NoOps.
_NO_HOIST_TYPES = frozenset({"InstNoOp"})


def _pair_ldweights(nc):
    """Reorder LDW0,MM0,LDW1,MM1 -> LDW0,LDW1,MM0,MM1 for row-group pairs.

    When the second weight load targets PE rows 64-127 while the first
    matmul only occupies rows 0-63, the loads run concurrently on disjoint
    sub-arrays and both matmul streams overlap, instead of serializing the
    second load behind the first stream.
    """
    for f in nc.m.functions:
        for blk in f.blocks:
            insts = blk.instructions
            changed = False
            i = 0
            while i + 3 < len(insts):
                a, b, c, d = insts[i:i + 4]
                if (
                    type(a).__name__ == "InstLdweights"
                    and type(b).__name__ == "InstMatmult"
                    and type(c).__name__ == "InstLdweights"
                    and type(d).__name__ == "InstMatmult"
                    and b.tile_position is not None
                    and c.tile_position is not None
                    and b.tile_position[0] == 0
                    and c.tile_position[0] == 64
                    and b.tile_size is not None
                    and b.tile_size[0] <= 64
                ):
                    insts[i + 1], insts[i + 2] = c, b
                    changed = True
                    i += 4
                else:
                    i += 1
            if changed:
                blk.instructions = insts


def _legalize_waits(nc):
    """Hoist multi-waits off engine instructions onto preceding NoOps.

    Most walrus instruction encodings (S3_LW matmul, DMA, ACT, DVE, drain)
    only have room for a single sync-wait command; Tile freely attaches
    several. Waits execute on the engine's sequencer in program order, so
    moving them to immediately-preceding NoOps is semantics-preserving.
    """
    import bass_rust

    for f in nc.m.functions:
        for blk in f.blocks:
            out = []
            changed = False
            for inst in blk.instructions:
                si = getattr(inst, "sync_info", None)
                if (
                    type(inst).__name__ not in _NO_HOIST_TYPES
                    and si is not None
                    and len(si.on_wait) >= 2
                ):
                    waits = list(si.on_wait)
                    for k, w in enumerate(waits[:-1]):
                        nop = bass_rust.InstNoOp(name=f"{inst.name}_hoistw{k}")
                        nop.engine = inst.engine
                        nop.sync_info = bass_rust.SyncInfo(
                            on_wait=[w], on_update=[]
                        )
                        out.append(nop)
                    si.on_wait = [waits[-1]]
                    changed = True
                out.append(inst)
            if changed:
                blk.instructions = out


def _build_program():
    import concourse.bass as bass
    import concourse.mybir as mybir
    import concourse.tile as tile

    nc = bass.Bass("TRN2", target_bir_lowering=False, debug=False, num_devices=NCORES)
    xT = nc.dram_tensor("xT", [D, T], mybir.dt.bfloat16, kind="ExternalInput").ap()
    w6 = nc.dram_tensor("w6", [D, 6 * HS], mybir.dt.bfloat16, kind="ExternalInput").ap()
    out = nc.dram_tensor("outR", [HPC * (HS + 1), T], mybir.dt.float32, kind="ExternalOutput").ap()

    with tile.TileContext(nc) as tc:
        _emit(tc, nc, xT, w6, out)
    _pair_ldweights(nc)
    _legalize_waits(nc)
    return nc


def _in_maps(x, Wk, Wq, Wv):
    import ml_dtypes

    bf = ml_dtypes.bfloat16
    xTh = np.ascontiguousarray(np.asarray(x, dtype=np.float32).T.astype(bf))
    maps = []
    for c in range(NCORES):
        h0, h1 = HPC * c, HPC * c + 1
        W6 = np.concatenate(
            [Wq[h0], Wq[h1], Wk[h0], Wk[h1], Wv[h0], Wv[h1]], axis=1
        ).astype(bf)
        maps.append({"xT": xTh, "w6": np.ascontiguousarray(W6)})
    return maps


def get_program():
    global _cached_nc
    if _cached_nc is None:
        _cached_nc = _build_program()
    return _cached_nc


def kernel(x, Wk, Wq, Wv):
    from concourse.bass_utils import run_bass_kernel_spmd

    nc = get_program()
    res = run_bass_kernel_spmd(nc, _in_maps(x, Wk, Wq, Wv), core_ids=list(range(NCORES)))
    cols = []
    for c in range(NCORES):
        raw = res.results[c]["outR"]  # [2*65, T]: per head 64 rows O^T + denom
        for h in range(HPC):
            o = raw[h * 65:h * 65 + HS]
            den = raw[h * 65 + HS:h * 65 + HS + 1]
            cols.append((o / den).T)
    return np.ascontiguousarray(np.concatenate(cols, axis=1), dtype=np.float32)


# revision 21
# speedup vs baseline: 1.0324x; 1.0269x over previous
"""Multi-head causal attention (scores = K @ Q^T variant) on 8 TRN2 NeuronCores.

Head-parallel sharding: core c computes heads (2c, 2c+1) end-to-end and the
host concatenates the per-core [T, 128] outputs along the feature axis.

Per-core kernel layout notes:
  - Host passes x transposed ([D, T]) and pre-cast to bf16 so every
    projection matmul has the contraction dim (d) on SBUF partitions with
    zero on-device transposes.
  - Q^T/K^T are stored [128, T] with head0 on partitions 0-63 and head1 on
    64-127, letting the S^T matmuls for both heads run concurrently on
    disjoint PE row-groups (tile_position).
  - Scores are computed transposed (S^T[j, i] = Q_j . K_i) so that the AV
    contraction (over j) lands on the partition axis with no transposes.
  - V is stored in natural layout with a fused ones-column ([V | 1]) so a
    single AV matmul produces both the weighted sum and the softmax
    denominator (PSUM row 64).
  - Softmax skips the max-subtraction (scores are ~N(0,1); exp is safe in
    fp32) which matches jax.nn.softmax up to rounding.
  - Matmul operands are bf16 (1 cycle/row on the PE; fp32 streams at ~2);
    PSUM accumulation and the final normalization stay fp32.
  - Projection chunk k and attention i-block k are emitted interleaved
    (i-block k only needs x columns < 512*(k+1)) from one pool set sized to
    exactly 8 PSUM banks, so projections and attention overlap instead of
    serializing on a pool boundary.
"""

import numpy as np

T, D, H, HS = 4096, 1024, 16, 64
NCORES = 8
HPC = H // NCORES  # heads per core = 2
DC = D // 128      # 8 contraction chunks
TC = T // 512      # 8 t-chunks for projections
IB = T // 512      # 8 i-blocks (512 output rows each)
JBN = T // 128     # 32 j-blocks (128 keys each)

_cached_nc = None


def _emit(tc, nc, xT, w6, out):
    import concourse.bass as bass  # noqa: F401
    import concourse.mybir as mybir

    f32 = mybir.dt.float32
    bf16 = mybir.dt.bfloat16
    Exp = mybir.ActivationFunctionType.Exp
    ne = mybir.AluOpType.not_equal
    ge = mybir.AluOpType.is_ge

    with (
        tc.tile_pool(name="const", bufs=1) as constp,
        tc.tile_pool(name="wpool", bufs=1) as wpool,
        tc.tile_pool(name="bigp", bufs=1) as bigp,
        tc.tile_pool(name="xpool", bufs=3) as xpool,
        tc.tile_pool(name="vtp", bufs=2) as vtp,
        tc.tile_pool(name="esp", bufs=6) as esp,
        tc.tile_pool(name="finp", bufs=4) as finp,
        # PSUM budget (8 banks total): s 2x2 + o 2x1 + p 2x1 (shared with
        # the V-transpose tiles so projections get double-buffering).
        tc.tile_pool(name="sp", bufs=2, space="PSUM") as sp,
        tc.tile_pool(name="op", bufs=2, space="PSUM") as op,
        tc.tile_pool(name="pp", bufs=2, space="PSUM") as pp,
    ):
        # ---- input DMAs for weights + first x chunk go first ------------
        w6r = w6.rearrange("(dc p) f -> p dc f", p=128)
        xTr = xT.rearrange("(dc p) t -> p dc t", p=128)
        w6sb = wpool.tile([128, DC, 6 * HS], bf16)
        xts = []
        xt0 = xpool.tile([128, DC, 512], bf16, tag="xt", name="xt0")
        # interleave so the first projection's (w6 dc0, xt0 dc0) land first;
        # the first x chunk stays per-dc so matmuls start on partial data.
        nc.sync.dma_start(out=w6sb[:, 0, :], in_=w6r[:, 0, :])
        nc.sync.dma_start(out=xt0[:, 0, :], in_=xTr[:, 0, 0:512])
        nc.sync.dma_start(out=w6sb[:, 1:DC, :], in_=w6r[:, 1:DC, :])
        for dc in range(1, DC):
            nc.sync.dma_start(out=xt0[:, dc, :], in_=xTr[:, dc, 0:512])
        xts.append(xt0)

        # ---- constants (gpsimd; overlaps the DMAs) ----------------------
        # id64: two stacked 64x64 identities so both head slices (partition
        # offset 0 and 64) see an identity for the V transposes.
        id64 = constp.tile([128, 64], bf16)
        nc.gpsimd.memset(id64, 0.0)
        nc.gpsimd.affine_select(
            out=id64, in_=id64, compare_op=ne, fill=1.0,
            base=0, channel_multiplier=1, pattern=[[-1, 64]],
        )
        nc.gpsimd.affine_select(
            out=id64, in_=id64, compare_op=ne, fill=1.0,
            base=-64, channel_multiplier=1, pattern=[[-1, 64]],
        )
        # Causal masks for the 4 diagonal block offsets: keep iff il >= jl + 128*q.
        mask4 = constp.tile([128, 4, 512], bf16)
        for q in range(4):
            nc.gpsimd.memset(mask4[:, q, :], 1.0)
            nc.gpsimd.affine_select(
                out=mask4[:, q, :], in_=mask4[:, q, :], compare_op=ge, fill=0.0,
                base=-128 * q, channel_multiplier=-1, pattern=[[1, 512]],
            )

        # ---- persistent activations ------------------------------------
        QT = bigp.tile([128, T], bf16)   # head0 rows 0-63, head1 rows 64-127
        KT = bigp.tile([128, T], bf16)
        Vext0 = bigp.tile([128, JBN, HS + 1], bf16)
        Vext1 = bigp.tile([128, JBN, HS + 1], bf16)
        onesb = constp.tile([128, JBN], bf16)
        nc.gpsimd.memset(onesb, 1.0)
        nc.vector.tensor_copy(Vext0[:, :, HS], onesb)
        nc.vector.tensor_copy(Vext1[:, :, HS], onesb)

        def emit_proj_chunk(tcj):
            ts = slice(tcj * 512, (tcj + 1) * 512)
            if tcj + 1 < TC:  # prefetch next x chunk (batched 3D DMAs)
                nxt = slice((tcj + 1) * 512, (tcj + 2) * 512)
                xtn = xpool.tile([128, DC, 512], bf16, tag="xt", name=f"xt{tcj + 1}")
                nc.sync.dma_start(out=xtn[:, 0:4, :], in_=xTr[:, 0:4, nxt])
                nc.sync.dma_start(out=xtn[:, 4:8, :], in_=xTr[:, 4:8, nxt])
                xts.append(xtn)
            xt = xts[tcj]
            for fc, dest in ((0, QT), (1, KT)):
                ps = pp.tile([128, 512], f32, tag="p", name=f"ps{fc}_{tcj}")
                for dc in range(DC):
                    nc.tensor.matmul(
                        ps,
                        lhsT=w6sb[:, dc, fc * 128:(fc + 1) * 128],
                        rhs=xt[:, dc, :],
                        start=(dc == 0), stop=(dc == DC - 1),
                    )
                nc.vector.tensor_copy(dest[:, ts], ps)
            psv = pp.tile([128, 512], f32, tag="p", name=f"psv_{tcj}")
            for dc in range(DC):
                nc.tensor.matmul(
                    psv,
                    lhsT=w6sb[:, dc, 256:384],
                    rhs=xt[:, dc, :],
                    start=(dc == 0), stop=(dc == DC - 1),
                )
            vts = vtp.tile([128, 512], bf16, tag="vts", name=f"vts_{tcj}")
            nc.vector.tensor_copy(vts, psv)
            for q in range(4):
                for h in range(HPC):
                    vdst = Vext0 if h == 0 else Vext1
                    ptv = pp.tile([128, 64], bf16, tag="p", name=f"ptv{h}_{tcj}_{q}")
                    nc.tensor.transpose(
                        ptv,
                        in_=vts[h * 64:(h + 1) * 64, q * 128:(q + 1) * 128],
                        identity=id64[h * 64:(h + 1) * 64, :],
                        tile_position=(h * 64, 0),
                    )
                    nc.vector.tensor_copy(vdst[:, tcj * 4 + q, 0:HS], ptv)

        def emit_attn_block(ib):
            isl = slice(ib * 512, (ib + 1) * 512)
            njb = 4 * (ib + 1)
            po = [
                op.tile([65, 512], f32, tag="o", name=f"po{h}_{ib}")
                for h in range(HPC)
            ]
            pending = None
            for jb in range(njb):
                ps = sp.tile([128, 2, 512], f32, tag="s", name=f"s_{ib}_{jb}")
                for h in range(HPC):
                    nc.tensor.matmul(
                        ps[:, h, :],
                        lhsT=QT[h * 64:(h + 1) * 64, jb * 128:(jb + 1) * 128],
                        rhs=KT[h * 64:(h + 1) * 64, isl],
                        start=True, stop=True,
                        tile_position=(h * 64, 0),
                    )
                es = esp.tile([128, 2, 512], bf16, tag="es", name=f"es_{ib}_{jb}")
                nc.scalar.activation(es, ps, Exp, scale=float(1.0 / np.sqrt(HS)))
                q = jb - 4 * ib
                if q >= 0:  # diagonal block: zero out j > i entries
                    for h in range(HPC):
                        nc.vector.tensor_mul(es[:, h, :], es[:, h, :], mask4[:, q, :])
                if pending is not None:
                    pjb, pes = pending
                    for h in range(HPC):
                        nc.tensor.matmul(
                            po[h],
                            lhsT=Vext0[:, pjb, :] if h == 0 else Vext1[:, pjb, :],
                            rhs=pes[:, h, :],
                            start=(pjb == 0), stop=False,
                        )
                pending = (jb, es)
            pjb, pes = pending
            for h in range(HPC):
                nc.tensor.matmul(
                    po[h],
                    lhsT=Vext0[:, pjb, :] if h == 0 else Vext1[:, pjb, :],
                    rhs=pes[:, h, :],
                    start=(pjb == 0), stop=True,
                )
            # evacuate the unnormalized O^T + denominator row; the host does
            # the (tiny) divide and the un-transpose during the gather.
            for h in range(HPC):
                ot = finp.tile([65, 512], f32, tag="ot", name=f"ot{h}_{ib}")
                nc.vector.tensor_copy(ot, po[h])
                nc.sync.dma_start(out=out[h * 65:(h + 1) * 65, isl], in_=ot)

        # Staircase: attention block k only depends on projection chunks <= k.
        for k in range(TC):
            emit_proj_chunk(k)
            emit_attn_block(k)


# walrus engine-instruction encodings have a single sync-wait slot; hoist
# extra waits onto per-wait NoOps for everything except generated NoOps.
_NO_HOIST_TYPES = frozenset({"InstNoOp"})


def _pair_ldweights(nc):
    """Reorder LDW0,MM0,LDW1,MM1 -> LDW0,LDW1,MM0,MM1 for row-group pairs.

    When the second weight load targets PE rows 64-127 while the first
    matmul only occupies rows 0-63, the loads run concurrently on disjoint
    sub-arrays and both matmul streams overlap, instead of serializing the
    second load behind the first stream.
    """
    for f in nc.m.functions:
        for blk in f.blocks:
            insts = blk.instructions
            changed = False
            i = 0
            while i + 3 < len(insts):
                a, b, c, d = insts[i:i + 4]
                if (
                    type(a).__name__ == "InstLdweights"
                    and type(b).__name__ == "InstMatmult"
                    and type(c).__name__ == "InstLdweights"
                    and type(d).__name__ == "InstMatmult"
                    and b.tile_position is not None
                    and c.tile_position is not None
                    and b.tile_position[0] == 0
                    and c.tile_position[0] == 64
                    and b.tile_size is not None
                    and b.tile_size[0] <= 64
                ):
                    insts[i + 1], insts[i + 2] = c, b
                    changed = True
                    i += 4
                else:
                    i += 1
            if changed:
                blk.instructions = insts


def _legalize_waits(nc):
    """Hoist multi-waits off engine instructions onto preceding NoOps.

    Most walrus instruction encodings (S3_LW matmul, DMA, ACT, DVE, drain)
    only have room for a single sync-wait command; Tile freely attaches
    several. Waits execute on the engine's sequencer in program order, so
    moving them to immediately-preceding NoOps is semantics-preserving.
    """
    import bass_rust

    for f in nc.m.functions:
        for blk in f.blocks:
            out = []
            changed = False
            for inst in blk.instructions:
                si = getattr(inst, "sync_info", None)
                if (
                    type(inst).__name__ not in _NO_HOIST_TYPES
                    and si is not None
                    and len(si.on_wait) >= 2
                ):
                    waits = list(si.on_wait)
                    for k, w in enumerate(waits[:-1]):
                        nop = bass_rust.InstNoOp(name=f"{inst.name}_hoistw{k}")
                        nop.engine = inst.engine
                        nop.sync_info = bass_rust.SyncInfo(
                            on_wait=[w], on_update=[]
                        )
                        out.append(nop)
                    si.on_wait = [waits[-1]]
                    changed = True
                out.append(inst)
            if changed:
                blk.instructions = out


def _build_program():
    import concourse.bass as bass
    import concourse.mybir as mybir
    import concourse.tile as tile

    nc = bass.Bass("TRN2", target_bir_lowering=False, debug=False, num_devices=NCORES)
    xT = nc.dram_tensor("xT", [D, T], mybir.dt.bfloat16, kind="ExternalInput").ap()
    w6 = nc.dram_tensor("w6", [D, 6 * HS], mybir.dt.bfloat16, kind="ExternalInput").ap()
    out = nc.dram_tensor("outR", [HPC * (HS + 1), T], mybir.dt.float32, kind="ExternalOutput").ap()

    with tile.TileContext(nc) as tc:
        _emit(tc, nc, xT, w6, out)
    _pair_ldweights(nc)
    _legalize_waits(nc)
    return nc


def _in_maps(x, Wk, Wq, Wv):
    import ml_dtypes

    bf = ml_dtypes.bfloat16
    xTh = np.ascontiguousarray(np.asarray(x, dtype=np.float32).T.astype(bf))
    maps = []
    for c in range(NCORES):
        h0, h1 = HPC * c, HPC * c + 1
        W6 = np.concatenate(
            [Wq[h0], Wq[h1], Wk[h0], Wk[h1], Wv[h0], Wv[h1]], axis=1
        ).astype(bf)
        maps.append({"xT": xTh, "w6": np.ascontiguousarray(W6)})
    return maps


def get_program():
    global _cached_nc
    if _cached_nc is None:
        _cached_nc = _build_program()
    return _cached_nc


def kernel(x, Wk, Wq, Wv):
    import os

    from concourse.bass_utils import run_bass_kernel_spmd

    # The neuronx-cc compile cache keys on tensor shapes only (not BIR
    # content), so a shared cache can serve a stale NEFF for a same-shape
    # program. Force a fresh compile; repeat calls in one process still hit
    # the in-memory jit cache.
    os.environ.setdefault("NEURON_FORCE_RECOMPILE", "1")

    nc = get_program()
    res = run_bass_kernel_spmd(nc, _in_maps(x, Wk, Wq, Wv), core_ids=list(range(NCORES)))
    cols = []
    for c in range(NCORES):
        raw = res.results[c]["outR"]  # [2*65, T]: per head 64 rows O^T + denom
        for h in range(HPC):
            o = raw[h * 65:h * 65 + HS]
            den = raw[h * 65 + HS:h * 65 + HS + 1]
            cols.append((o / den).T)
    return np.ascontiguousarray(np.concatenate(cols, axis=1), dtype=np.float32)
